# revision 25
# baseline (speedup 1.0000x reference)
"""Trainium2 Bass kernel for GQA attention with RoPE (tensor-parallel over heads).

Reference computation (per problem spec):
  x:[1,2048,4096], wq:[4096,4096], wk/wv:[4096,1024], wo:[4096,4096], f32
  q/k/v proj -> RoPE(q,k) -> causal GQA softmax attention -> o_proj

Sharding: 8 cores, tensor-parallel over heads. Core c gets 4 query heads
(wq cols [c*512:(c+1)*512]) and 1 KV head (wk/wv cols [c*128:(c+1)*128]),
plus wo rows [c*512:(c+1)*512]. Each core computes a full [2048,4096]
partial o_proj output; the host sums the 8 partials (the all-reduce).
The host dispatch layer hands the device x pre-transposed ([D,S]) -- the
TensorE contracts over the partition axis, so both matmul operands need
d on partitions.

Matmul operands are fp16 (see previous-gen docstring: FWL hides weight
loads under the 1-col/cycle matmul stream; fp32 pays serialized 2-pass
LDWEIGHTS, fp8 DoubleRow fails the 2e-2 accuracy gate by 2-5x). All
accumulation is fp32 in PSUM.

Phase 2 is a single flattened software-pipelined job stream: each job is
one (q-tile, head, key-chunk-pair) -> 2 score MMs [128x128x256] into one
PSUM bank, one exp ACT, (diagonal-only) causal mask multiply on DVE, and
2 PV MMs lagging the scores by PIPE_LAG jobs so the exp latency hides
under later jobs' score MMs. Softmax denominators are accumulated on
VectorE (pacc += exp chunks, fp16) instead of the old per-chunk
ones-vector matmuls (which cost 295ns each of pure TensorE time and
broke FWL for the neighboring PV matmuls); one [128x128x256] matmul
against an all-ones stationary both reduces pacc over partitions and
broadcasts the result to all 128 partitions, replacing the old
GpSimd partition_broadcast. o_proj mi-blocks are interleaved one per
job into the stream so TensorE always has dense independent work while
ScalarE catches up on exps.
"""
import numpy as np
from collections import deque

import concourse.bass as bass
import concourse.bacc as bacc
import concourse.tile as tile
import concourse.mybir as mybir
from concourse import bass_utils

F32 = mybir.dt.float32
F16 = mybir.dt.float16
AF = mybir.ActivationFunctionType

# model dims (hardcoded per problem spec nn_Attention_52020643889298)
S = 2048
D = 4096
H = 32
KV = 8
HD = 128
THETA = 10000.0
NCORES = 8
HQ = H // NCORES            # 4 query heads per core
NQ = HQ * HD                # 512 wq cols per core
NKV = (KV // NCORES) * HD   # 128 wk/wv cols per core

# tiling
SSTRIP = 512                # phase-1 s-strip
NSTRIPS = S // SSTRIP       # 4
NSUB = SSTRIP // 128        # 4
DCH = D // 128              # 32 contraction chunks
QTILE = 256                 # attention q-tile
NQT = S // QTILE            # 8
NPCH = S // 128             # 16 key chunks

EXP_BIAS = -10.0            # exp(s-10): keeps exp in fp16 range; cancels
                            # in the softmax normalization
PIPE_LAG = 2                # PV lags scores by this many jobs
ST_BUFS = 3                 # score PSUM ring (1 bank each at QTILE=256)


def _rope_tables():
    inv = 1.0 / (THETA ** (np.arange(0, HD, 2, dtype=np.float64) / HD))
    pos = np.arange(S, dtype=np.float64)
    freqs = pos[:, None] * inv[None, :]          # [S, 64]
    emb = np.concatenate([freqs, freqs], axis=1)  # [S, HD]
    cosT = np.cos(emb).T.astype(np.float16).copy()  # [HD, S]
    sinT = np.sin(emb).T.astype(np.float16).copy()
    return cosT, sinT


def _mask_pair():
    # One key-chunk pair (256 keys) against QTILE queries on the diagonal:
    # chunk 0 keys are at local offset p, chunk 1 at p+128.
    # mask[p, c*QTILE + q'] = 1 iff q' - p >= 128*c
    q = np.arange(QTILE)[None, :]
    p = np.arange(128)[:, None]
    cols = [np.where(q - p >= 128 * c, 1.0, 0.0) for c in range(2)]
    return np.concatenate(cols, axis=1).astype(np.float16)  # [128, 512]


def build():
    nc = bacc.Bacc("TRN2", target_bir_lowering=False, debug=False,
                   enable_asserts=False, num_devices=NCORES)
    xt_d = nc.dram_tensor("xt", [D, S], F16, kind="ExternalInput").ap()
    wq_d = nc.dram_tensor("wq", [D, NQ], F16, kind="ExternalInput").ap()
    wk_d = nc.dram_tensor("wk", [D, NKV], F16, kind="ExternalInput").ap()
    wv_d = nc.dram_tensor("wv", [D, NKV], F16, kind="ExternalInput").ap()
    wo_d = nc.dram_tensor("wo", [NQ, D], F16, kind="ExternalInput").ap()
    out_d = nc.dram_tensor("out", [S, D], F16, kind="ExternalOutput").ap()

    cosT, sinT = _rope_tables()
    allconst = np.concatenate(
        [np.eye(128, dtype=np.float16), cosT, sinT, _mask_pair(),
         np.ones((128, 128), dtype=np.float16)], axis=1)
    const_d = nc.inline_tensor(allconst, "allconst").ap()

    with tile.TileContext(nc) as tc:
        _body(nc, tc, xt_d, wq_d, wk_d, wv_d, wo_d, out_d, const_d)
    nc.compile()
    return nc


def _body(nc, tc, xt_d, wq_d, wk_d, wv_d, wo_d, out_d, const_d):
    wqr = wq_d.rearrange("(c p) n -> p c n", p=128)
    wkr = wk_d.rearrange("(c p) n -> p c n", p=128)
    wvr = wv_d.rearrange("(c p) n -> p c n", p=128)

    with tc.tile_pool(name="const", bufs=1) as const_pool, \
         tc.tile_pool(name="persist", bufs=1) as persist:

        # persistent activations; qT/kT are split per strip so phase-2's
        # first jobs (which only need strip 0) don't inherit a dependency
        # on the last strip's RoPE drains
        qT_t = [persist.tile([128, HQ, SSTRIP], F16, name=f"qT{i}")
                for i in range(NSTRIPS)]
        kT_t = [persist.tile([128, SSTRIP], F16, name=f"kT{i}")
                for i in range(NSTRIPS)]
        vnat_sb = persist.tile([128, NPCH, HD], F16)  # [s%128, s//128, hd]

        # ---------------- phase 1: QKV projection + RoPE ----------------
        wo_pool_cm = tc.tile_pool(name="wo2", bufs=1)
        outh_pool_cm = tc.tile_pool(name="outh", bufs=1)
        wo_pool = wo_pool_cm.__enter__()
        outh_pool = outh_pool_cm.__enter__()
        wo_sb = wo_pool.tile([128, HQ, D], F16)
        outhT_sb = outh_pool.tile([128, HQ, S], F16)  # [hd, head, s]
        with tc.tile_pool(name="w1", bufs=1) as w1, \
             tc.tile_pool(name="xt", bufs=1) as xt_pool, \
             tc.tile_pool(name="p1tmp", bufs=2) as p1tmp, \
             tc.tile_pool(name="tp_ps", bufs=2, space="PSUM") as tp_ps, \
             tc.tile_pool(name="acc_ps", bufs=1, space="PSUM") as acc_ps:

            wq_sb = w1.tile([128, DCH, NQ], F16)
            wk_sb = w1.tile([128, DCH, NKV], F16)
            wv_sb = w1.tile([128, DCH, NKV], F16)

            xtr = xt_d.rearrange("(c p) s -> p c s", p=128)  # [128, DCH, S]

            # Every dma_start costs the Sync engine ~1us of serial SWDGE
            # descriptor-generation time regardless of size, so DMAs are
            # few and large, issued in consumption order: strip-0 x and wq
            # arrive as graduated ranges (small first for latency), wk/wv
            # in two pieces, later strips as two half-strip tiles
            # prefetched one strip ahead.
            xstore = {}   # si -> list of (tile, lo, hi)

            def load_xt_range(si, lo, hi, tag, bufs=1):
                t = xt_pool.tile([128, hi - lo, SSTRIP], F16, tag=tag,
                                 name=f"xt{si}_{lo}", bufs=bufs)
                nc.sync.dma_start(
                    t[:], xtr[:, lo:hi,
                              si * SSTRIP:(si + 1) * SSTRIP])
                xstore.setdefault(si, []).append((t, lo, hi))

            def load_half(si, half):
                load_xt_range(si, 16 * half, 16 * half + 16, "xtbig", bufs=3)

            # interleave strip-0 x and weights in dc order; strip-1 x
            # rides behind them so its sweeps never wait
            load_xt_range(0, 0, 4, "xs0a")
            nc.sync.dma_start(wq_sb[:, 0:4, :], wqr[:, 0:4, :])
            nc.sync.dma_start(wk_sb[:, 0:8, :], wkr[:, 0:8, :])
            nc.sync.dma_start(wv_sb[:, 0:8, :], wvr[:, 0:8, :])
            load_xt_range(0, 4, 16, "xs0b")
            nc.sync.dma_start(wq_sb[:, 4:10, :], wqr[:, 4:10, :])
            nc.sync.dma_start(wq_sb[:, 10:16, :], wqr[:, 10:16, :])
            nc.sync.dma_start(wk_sb[:, 8:32, :], wkr[:, 8:32, :])
            nc.sync.dma_start(wv_sb[:, 8:32, :], wvr[:, 8:32, :])
            load_half(0, 1)
            nc.sync.dma_start(wq_sb[:, 16:32, :], wqr[:, 16:32, :])
            load_half(1, 0)
            load_half(1, 1)

            allc = const_pool.tile([128, 4864], F16)
            nc.sync.dma_start(allc[:], const_d[:])
            ident = allc[:, 0:128]
            COS0, SIN0 = 128, 2176
            mask_sb = allc[:, 4224:4736]
            ones_sb = allc[:, 4736:4864]
            ebias = const_pool.tile([128, 1], F32)
            nc.gpsimd.memset(ebias[:], EXP_BIAS)
            # warm the ScalarE exp table during phase 1 (ACT_TABLE_LOAD is
            # ~1.3us and otherwise lands on the phase-2 critical path)
            expwarm = const_pool.tile([128, 1], F32)
            nc.scalar.activation(expwarm[:], ebias[:], AF.Exp, bias=ebias[:])

            def rope_store(src_ps, dst_ap, sslice):
                # dst = src*cos + rot(src)*sin, rot = [-src[64:], src[:64]].
                # SBUF+SBUF DVE operands must share their base partition, so
                # materialize the half-rotated src from PSUM first, then all
                # remaining ops are partition-aligned fp16 SBUF math.
                qrot = p1tmp.tile([128, SSTRIP], F16, tag="rope_qr",
                                  name="rope_qr")
                nc.vector.tensor_copy(qrot[0:64, :], src_ps[64:128, :])
                nc.vector.tensor_copy(qrot[64:128, :], src_ps[0:64, :])
                qcos = p1tmp.tile([128, SSTRIP], F16, tag="rope_qc",
                                  name="rope_qc")
                s0 = sslice.start
                nc.vector.tensor_mul(qcos[:], src_ps[:],
                                     allc[:, COS0 + s0:COS0 + s0 + SSTRIP])
                nc.vector.tensor_mul(qrot[:], qrot[:],
                                     allc[:, SIN0 + s0:SIN0 + s0 + SSTRIP])
                nc.vector.tensor_sub(dst_ap[0:64, :], qcos[0:64, :],
                                     qrot[0:64, :])
                nc.vector.tensor_add(dst_ap[64:128, :], qcos[64:128, :],
                                     qrot[64:128, :])

            def vtranspose(si, vacc):
                vstg = p1tmp.tile([128, SSTRIP], F16, tag="vstg",
                                  name="vstg")
                nc.scalar.copy(vstg[:], vacc[:])
                for ss in range(NSUB):
                    tp = tp_ps.tile([128, 128], F16, tag="tp", name="tp")
                    nc.tensor.transpose(
                        tp[:], vstg[:, ss * 128:(ss + 1) * 128], ident[:])
                    nc.vector.tensor_copy(vnat_sb[:, si * NSUB + ss, :],
                                          tp[:])

            for si in range(NSTRIPS):
                s0 = si * SSTRIP
                sslice = slice(s0, s0 + SSTRIP)
                if 1 <= si < NSTRIPS - 1:
                    # next strip's low half now; its high half at strip
                    # end (the 3-deep ring slot frees once this strip's
                    # low-half reads are all emitted)
                    load_half(si + 1, 0)
                if si == 3:
                    # prefetch wo for o_proj; deferred past the phase-1
                    # weight/x burst so it never competes with them
                    nc.sync.dma_start(wo_sb[:],
                                      wo_d.rearrange("(c p) m -> p c m",
                                                     p=128))

                qacc = [acc_ps.tile([128, SSTRIP], F32, tag=f"qacc{g}",
                                    name=f"qacc{g}")
                        for g in range(HQ)]
                kacc = acc_ps.tile([128, SSTRIP], F32, tag="kacc")
                vacc = acc_ps.tile([128, SSTRIP], F32, tag="vacc")

                xtiles = xstore.pop(si)

                def xchunk(dc):
                    for t, lo, hi in xtiles:
                        if lo <= dc < hi:
                            return t[:, dc - lo, :]
                    raise KeyError(dc)

                wsl_q = [
                    (lambda dc, g=g: wq_sb[:, dc, g * 128:(g + 1) * 128])
                    for g in range(HQ)]
                wsl_k = lambda dc: wk_sb[:, dc, :]
                wsl_v = lambda dc: wv_sb[:, dc, :]

                def mm(acc, wsl, dc):
                    nc.tensor.matmul(acc[:], wsl(dc), xchunk(dc),
                                     start=(dc == 0), stop=(dc == DCH - 1))

                if si == 0:
                    # dc-major warmup: consume x/w chunks in DMA arrival
                    # order so the PE never outruns the HBM stream during
                    # the cold start; finish output-major so the RoPE
                    # drains overlap the remaining matmuls.
                    DCSPLIT = 24
                    allacc = ([(qacc[g], wsl_q[g]) for g in range(HQ)]
                              + [(kacc, wsl_k), (vacc, wsl_v)])
                    for dc in range(DCSPLIT):
                        for acc, wsl in allacc:
                            mm(acc, wsl, dc)
                    for dc in range(DCSPLIT, DCH):
                        mm(vacc, wsl_v, dc)
                    vtranspose(si, vacc)
                    for dc in range(DCSPLIT, DCH):
                        mm(kacc, wsl_k, dc)
                    rope_store(kacc, kT_t[si][:], sslice)
                    for g in reversed(range(HQ)):
                        for dc in range(DCSPLIT, DCH):
                            mm(qacc[g], wsl_q[g], dc)
                        rope_store(qacc[g], qT_t[si][:, g, :], sslice)
                else:
                    # dc-major over the low half: all six accumulators
                    # advance together, so the strip's high-half x tile
                    # (whose DMA was issued only at the previous strip's
                    # end) isn't touched until ~21us into the strip.
                    # Then output-major k, v(+transposes), q3..q0 over the
                    # high half with drains trailing under later sweeps.
                    allacc = ([(kacc, wsl_k), (vacc, wsl_v)]
                              + [(qacc[g], wsl_q[g])
                                 for g in reversed(range(HQ))])
                    for dc in range(16):
                        for acc, wsl in allacc:
                            mm(acc, wsl, dc)
                    for dc in range(16, DCH):
                        mm(kacc, wsl_k, dc)
                    rope_store(kacc, kT_t[si][:], sslice)
                    if si < NSTRIPS - 1:
                        for dc in range(16, DCH):
                            mm(vacc, wsl_v, dc)
                        vtranspose(si, vacc)
                    for g in reversed(range(HQ)):
                        for dc in range(16, DCH):
                            mm(qacc[g], wsl_q[g], dc)
                        rope_store(qacc[g], qT_t[si][:, g, :], sslice)
                    if si == NSTRIPS - 1:
                        # v last: its 16 matmuls + transposes are TensorE
                        # work that covers all four q RoPE drains, so no
                        # phase-2 PSUM bank waits on a drain's reads
                        for dc in range(16, DCH):
                            mm(vacc, wsl_v, dc)
                        vtranspose(si, vacc)
                    if 1 <= si < NSTRIPS - 1:
                        load_half(si + 1, 1)

        # -------- phase 2: attention + o_proj, one pipelined stream --------
        # job = (qi, h, pp): one pair of key chunks (2*128 keys) against
        # q-tile qi (256 queries) for head h. npairs(qi) = qi+1.
        jobs = []
        for qi in range(NQT):
            for h in range(HQ):
                npairs = qi + 1
                for pp in range(npairs):
                    jobs.append((qi, h, pp, npairs))
        njobs = len(jobs)

        with tc.tile_pool(name="pt", bufs=4) as pt_pool, \
             tc.tile_pool(name="pacc", bufs=6) as pacc_pool, \
             tc.tile_pool(name="a2tmp", bufs=2) as a2tmp, \
             tc.tile_pool(name="osb", bufs=2) as osb_pool, \
             tc.tile_pool(name="st_ps", bufs=ST_BUFS, space="PSUM") as st_ps, \
             tc.tile_pool(name="oacc_ps", bufs=2, space="PSUM") as oacc_ps, \
             tc.tile_pool(name="rb_ps", bufs=1, space="PSUM") as rb_ps, \
             tc.tile_pool(name="opj_ps", bufs=2, space="PSUM") as opj_ps:

            st_tiles = {}    # j -> score PSUM tile
            pt_tiles = {}    # j -> exp'd prob tile (SBUF fp16)
            blk_state = {}   # (qi,h) -> dict(pacc=, oacc=)
            pending_tails = deque()   # ((qi,h), emit_at_job)
            pending_oproj = deque()   # (si, mi)
            osb_tiles = {}   # si -> osb tile

            def emit_scores(j):
                qi, h, pp, npairs = jobs[j]
                q0 = qi * QTILE
                st = st_ps.tile([128, 2 * QTILE], F32, tag="st", name="st")
                ql = (qi % 2) * QTILE
                for c in range(2):
                    pi = 2 * pp + c
                    kl = (pi % 4) * 128
                    nc.tensor.matmul(
                        st[:, c * QTILE:(c + 1) * QTILE],
                        kT_t[pi // 4][:, kl:kl + 128],
                        qT_t[qi // 2][:, h, ql:ql + QTILE],
                        start=True, stop=True)
                st_tiles[j] = st

            def emit_exp_acc(j):
                qi, h, pp, npairs = jobs[j]
                pt = pt_pool.tile([128, 2 * QTILE], F16, tag="pt", name="pt")
                nc.scalar.activation(pt[:], st_tiles.pop(j)[:], AF.Exp,
                                     bias=ebias[:])
                if pp == npairs - 1:
                    # final pair straddles the causal diagonal
                    nc.vector.tensor_mul(pt[:], pt[:], mask_sb[:])
                pt_tiles[j] = pt
                # denominator accumulation on DVE: one double-width add per
                # pair (even chunks land in cols [0,256), odd in [256,512));
                # the fold to per-query sums happens via the rb matmul + one
                # narrow add in the tail. Halves the serial chain per block.
                stt = blk_state[(qi, h)]
                if pp == 0:
                    pacc = pacc_pool.tile([128, 2 * QTILE], F16, tag="pacc",
                                          name="pacc")
                    nc.vector.tensor_copy(pacc[:], pt[:])
                    stt["pacc"] = pacc
                else:
                    pacc = stt["pacc"]
                    nc.vector.tensor_add(pacc[:], pacc[:], pt[:])

            def emit_pv(j):
                qi, h, pp, npairs = jobs[j]
                stt = blk_state[(qi, h)]
                if pp == 0:
                    stt["oacc"] = oacc_ps.tile([128, QTILE], F32, tag="oacc", name="oacc")
                oacc = stt["oacc"]
                pt = pt_tiles.pop(j)
                for c in range(2):
                    pi = 2 * pp + c
                    nc.tensor.matmul(
                        oacc[:], vnat_sb[:, pi, :],
                        pt[:, c * QTILE:(c + 1) * QTILE],
                        start=(pi == 0), stop=(pi == 2 * npairs - 1))

            def emit_tail(key):
                qi, h = key
                stt = blk_state.pop(key)
                q0 = qi * QTILE
                # two matmuls against an all-ones stationary reduce pacc
                # over partitions, fold the even/odd-chunk halves via PSUM
                # accumulation, and broadcast the sums to all 128 output
                # partitions
                rb = rb_ps.tile([128, QTILE], F32, tag="rb", name="rb")
                nc.tensor.matmul(rb[:], ones_sb[:], stt["pacc"][:, 0:QTILE],
                                 start=True, stop=False)
                nc.tensor.matmul(rb[:], ones_sb[:],
                                 stt["pacc"][:, QTILE:2 * QTILE],
                                 start=False, stop=True)
                rbr = a2tmp.tile([128, QTILE], F32, tag="rbr", name="rbr")
                nc.vector.reciprocal_approx_fast(rbr[:], rb[:])
                nc.vector.tensor_mul(outhT_sb[:, h, q0:q0 + QTILE],
                                     stt["oacc"][:], rbr[:])
                if h == HQ - 1:
                    for si in (2 * qi, 2 * qi + 1):
                        for mi in range(D // 512):
                            pending_oproj.append((si, mi))

            def emit_oproj_block():
                si, mi = pending_oproj.popleft()
                if mi == 0:
                    osb_tiles[si] = osb_pool.tile([128, D], F16, tag="osb",
                                                  name="osb")
                osb = osb_tiles[si]
                op = opj_ps.tile([128, 512], F32, tag="opj", name="opj")
                for h in range(HQ):
                    nc.tensor.matmul(
                        op[:],
                        outhT_sb[:, h, si * 128:(si + 1) * 128],
                        wo_sb[:, h, mi * 512:(mi + 1) * 512],
                        start=(h == 0), stop=(h == HQ - 1))
                if mi % 2 == 0:
                    nc.vector.tensor_copy(
                        osb[:, mi * 512:(mi + 1) * 512], op[:])
                else:
                    nc.scalar.copy(
                        osb[:, mi * 512:(mi + 1) * 512], op[:])
                # one output DMA per row-block (DMA triggers are ~1us of
                # serial Sync time each); the last block goes in halves so
                # its DMA overlaps the final copies
                if si == 2 * NQT - 1:
                    if mi == 3:
                        nc.sync.dma_start(
                            out_d[si * 128:(si + 1) * 128, 0:2048],
                            osb[:, 0:2048])
                    elif mi == 7:
                        half = osb_tiles.pop(si)
                        nc.sync.dma_start(
                            out_d[si * 128:(si + 1) * 128, 2048:4096],
                            half[:, 2048:4096])
                elif mi == 7:
                    full = osb_tiles.pop(si)
                    nc.sync.dma_start(out_d[si * 128:(si + 1) * 128, :],
                                      full[:])

            for j in range(njobs + PIPE_LAG):
                if j < njobs:
                    qi, h, pp, npairs = jobs[j]
                    if pp == 0:
                        blk_state[(qi, h)] = {}
                    emit_scores(j)
                    emit_exp_acc(j)
                jl = j - PIPE_LAG
                if jl >= 0:
                    qi, h, pp, npairs = jobs[jl]
                    emit_pv(jl)
                    if pp == npairs - 1:
                        due = j + (2 if h == HQ - 1 else 1)
                        pending_tails.append(((qi, h), due))
                if pending_oproj and (len(pending_oproj) > 4
                                      or j % 2 == 0):
                    # pace the drain so dense o_proj work lasts through the
                    # ScalarE-limited attention-only stretches
                    emit_oproj_block()
                while pending_tails and pending_tails[0][1] <= j:
                    emit_tail(pending_tails.popleft()[0])
            # drain remaining tails and o_proj blocks
            while pending_tails:
                emit_tail(pending_tails.popleft()[0])
            while pending_oproj:
                emit_oproj_block()
        outh_pool_cm.__exit__(None, None, None)
        wo_pool_cm.__exit__(None, None, None)


_NC_CACHE = None
LAST_RESULT = None
RUN_KWARGS = {}


def _get_nc():
    global _NC_CACHE
    if _NC_CACHE is None:
        _NC_CACHE = build()
    return _NC_CACHE


def kernel(x, wq, wk, wv, wo):
    global LAST_RESULT
    x = np.asarray(x, dtype=np.float32).reshape(S, D)
    xt = np.ascontiguousarray(x.T.astype(np.float16))
    wq = (np.asarray(wq, dtype=np.float32)
          * np.float32(1.0 / np.sqrt(HD))).astype(np.float16)
    wk = np.asarray(wk, dtype=np.float32).astype(np.float16)
    wv = np.asarray(wv, dtype=np.float32).astype(np.float16)
    wo = np.asarray(wo, dtype=np.float32).astype(np.float16)

    in_maps = []
    for c in range(NCORES):
        in_maps.append({
            "xt": xt,
            "wq": np.ascontiguousarray(wq[:, c * NQ:(c + 1) * NQ]),
            "wk": np.ascontiguousarray(wk[:, c * NKV:(c + 1) * NKV]),
            "wv": np.ascontiguousarray(wv[:, c * NKV:(c + 1) * NKV]),
            "wo": np.ascontiguousarray(wo[c * NQ:(c + 1) * NQ, :]),
        })

    nc = _get_nc()
    res = bass_utils.run_bass_kernel_spmd(nc, in_maps,
                                          core_ids=list(range(NCORES)),
                                          **RUN_KWARGS)
    LAST_RESULT = res
    acc = np.zeros((S, D), dtype=np.float64)
    for c in range(NCORES):
        acc += res.results[c]["out"].astype(np.float64)
    return acc.astype(np.float32).reshape(1, S, D)


# revision 26
# speedup vs baseline: 1.1854x; 1.1854x over previous
"""Trainium2 Bass kernel for GQA attention with RoPE (tensor-parallel over heads).

Reference computation (per problem spec):
  x:[1,2048,4096], wq:[4096,4096], wk/wv:[4096,1024], wo:[4096,4096], f32
  q/k/v proj -> RoPE(q,k) -> causal GQA softmax attention -> o_proj

Sharding: 8 cores, tensor-parallel over heads. Core c gets 4 query heads
(wq cols [c*512:(c+1)*512]) and 1 KV head (wk/wv cols [c*128:(c+1)*128]),
plus wo rows [c*512:(c+1)*512]. Each core computes a full [2048,4096]
partial o_proj output; the host sums the 8 partials (the all-reduce).
The host dispatch layer hands the device x pre-transposed ([D,S]) -- the
TensorE contracts over the partition axis, so both matmul operands need
d on partitions.

Matmul operands are fp16 (see previous-gen docstring: FWL hides weight
loads under the 1-col/cycle matmul stream; fp32 pays serialized 2-pass
LDWEIGHTS, fp8 DoubleRow fails the 2e-2 accuracy gate by 2-5x). All
accumulation is fp32 in PSUM.

Phase 2 is a single flattened software-pipelined job stream: each job is
one (q-tile, head, key-chunk-pair) -> 2 score MMs [128x128x256] into one
PSUM bank, one exp ACT, (diagonal-only) causal mask multiply on DVE, and
2 PV MMs lagging the scores by PIPE_LAG jobs so the exp latency hides
under later jobs' score MMs. Softmax denominators are accumulated on
VectorE (pacc += exp chunks, fp16) instead of the old per-chunk
ones-vector matmuls (which cost 295ns each of pure TensorE time and
broke FWL for the neighboring PV matmuls); one [128x128x256] matmul
against an all-ones stationary both reduces pacc over partitions and
broadcasts the result to all 128 partitions, replacing the old
GpSimd partition_broadcast. o_proj mi-blocks are interleaved one per
job into the stream so TensorE always has dense independent work while
ScalarE catches up on exps.
"""
import numpy as np
from collections import deque

import concourse.bass as bass
import concourse.bacc as bacc
import concourse.tile as tile
import concourse.mybir as mybir
from concourse import bass_utils

F32 = mybir.dt.float32
F16 = mybir.dt.float16
AF = mybir.ActivationFunctionType

# model dims (hardcoded per problem spec nn_Attention_52020643889298)
S = 2048
D = 4096
H = 32
KV = 8
HD = 128
THETA = 10000.0
NCORES = 8
HQ = H // NCORES            # 4 query heads per core
NQ = HQ * HD                # 512 wq cols per core
NKV = (KV // NCORES) * HD   # 128 wk/wv cols per core

# tiling
SSTRIP = 512                # phase-1 s-strip
NSTRIPS = S // SSTRIP       # 4
NSUB = SSTRIP // 128        # 4
DCH = D // 128              # 32 contraction chunks
QTILE = 256                 # attention q-tile
NQT = S // QTILE            # 8
NPCH = S // 128             # 16 key chunks

EXP_BIAS = -10.0            # exp(s-10): keeps exp in fp16 range; cancels
                            # in the softmax normalization
PIPE_LAG = 2                # PV lags scores by this many jobs
ST_BUFS = 3                 # score PSUM ring (1 bank each at QTILE=256)


def _rope_tables():
    inv = 1.0 / (THETA ** (np.arange(0, HD, 2, dtype=np.float64) / HD))
    pos = np.arange(S, dtype=np.float64)
    freqs = pos[:, None] * inv[None, :]          # [S, 64]
    emb = np.concatenate([freqs, freqs], axis=1)  # [S, HD]
    cosT = np.cos(emb).T.astype(np.float16).copy()  # [HD, S]
    sinT = np.sin(emb).T.astype(np.float16).copy()
    return cosT, sinT


def _mask_pair():
    # One key-chunk pair (256 keys) against QTILE queries on the diagonal:
    # chunk 0 keys are at local offset p, chunk 1 at p+128.
    # mask[p, c*QTILE + q'] = 1 iff q' - p >= 128*c
    q = np.arange(QTILE)[None, :]
    p = np.arange(128)[:, None]
    cols = [np.where(q - p >= 128 * c, 1.0, 0.0) for c in range(2)]
    return np.concatenate(cols, axis=1).astype(np.float16)  # [128, 512]


def build():
    nc = bacc.Bacc("TRN2", target_bir_lowering=False, debug=False,
                   enable_asserts=False, num_devices=NCORES)
    xt_d = nc.dram_tensor("xt", [D, S], F16, kind="ExternalInput").ap()
    wq_d = nc.dram_tensor("wq", [D, NQ], F16, kind="ExternalInput").ap()
    wk_d = nc.dram_tensor("wk", [D, NKV], F16, kind="ExternalInput").ap()
    wv_d = nc.dram_tensor("wv", [D, NKV], F16, kind="ExternalInput").ap()
    wo_d = nc.dram_tensor("wo", [NQ, D], F16, kind="ExternalInput").ap()
    out_d = nc.dram_tensor("out", [S, D], F16, kind="ExternalOutput").ap()

    cosT, sinT = _rope_tables()
    allconst = np.concatenate(
        [np.eye(128, dtype=np.float16), cosT, sinT, _mask_pair(),
         np.ones((128, 128), dtype=np.float16)], axis=1)
    const_d = nc.inline_tensor(allconst, "allconst").ap()

    with tile.TileContext(nc) as tc:
        _body(nc, tc, xt_d, wq_d, wk_d, wv_d, wo_d, out_d, const_d)
    nc.compile()
    return nc


def _body(nc, tc, xt_d, wq_d, wk_d, wv_d, wo_d, out_d, const_d):
    wqr = wq_d.rearrange("(p c) n -> p c n", p=128)
    wkr = wk_d.rearrange("(p c) n -> p c n", p=128)
    wvr = wv_d.rearrange("(p c) n -> p c n", p=128)

    with tc.tile_pool(name="const", bufs=1) as const_pool, \
         tc.tile_pool(name="persist", bufs=1) as persist:

        # persistent activations; qT/kT are split per strip so phase-2's
        # first jobs (which only need strip 0) don't inherit a dependency
        # on the last strip's RoPE drains
        qT_t = [persist.tile([128, HQ, SSTRIP], F16, name=f"qT{i}")
                for i in range(NSTRIPS)]
        kT_t = [persist.tile([128, SSTRIP], F16, name=f"kT{i}")
                for i in range(NSTRIPS)]
        vnat_sb = persist.tile([128, NPCH, HD], F16)  # [s%128, s//128, hd]

        # ---------------- phase 1: QKV projection + RoPE ----------------
        wo_pool_cm = tc.tile_pool(name="wo2", bufs=1)
        outh_pool_cm = tc.tile_pool(name="outh", bufs=1)
        wo_pool = wo_pool_cm.__enter__()
        outh_pool = outh_pool_cm.__enter__()
        wo_sb = wo_pool.tile([128, HQ, D], F16)
        outhT_sb = outh_pool.tile([128, HQ, S], F16)  # [hd, head, s]
        with tc.tile_pool(name="w1", bufs=1) as w1, \
             tc.tile_pool(name="xt", bufs=1) as xt_pool, \
             tc.tile_pool(name="p1tmp", bufs=2) as p1tmp, \
             tc.tile_pool(name="tp_ps", bufs=2, space="PSUM") as tp_ps, \
             tc.tile_pool(name="acc_ps", bufs=1, space="PSUM") as acc_ps:

            wq_sb = w1.tile([128, DCH, NQ], F16)
            wk_sb = w1.tile([128, DCH, NKV], F16)
            wv_sb = w1.tile([128, DCH, NKV], F16)

            xtr = xt_d.rearrange("(p c) s -> p c s", p=128)  # [128, DCH, S]

            # Every dma_start costs the Sync engine ~1us of serial SWDGE
            # descriptor-generation time regardless of size, so DMAs are
            # few and large, issued in consumption order: strip-0 x and wq
            # arrive as graduated ranges (small first for latency), wk/wv
            # in two pieces, later strips as two half-strip tiles
            # prefetched one strip ahead.
            xstore = {}   # si -> list of (tile, lo, hi)

            def load_xt_range(si, lo, hi, tag, bufs=1):
                t = xt_pool.tile([128, hi - lo, SSTRIP], F16, tag=tag,
                                 name=f"xt{si}_{lo}", bufs=bufs)
                nc.sync.dma_start(
                    t[:], xtr[:, lo:hi,
                              si * SSTRIP:(si + 1) * SSTRIP])
                xstore.setdefault(si, []).append((t, lo, hi))

            def load_half(si, half):
                load_xt_range(si, 16 * half, 16 * half + 16, "xtbig", bufs=3)

            # interleave strip-0 x and weights in dc order; strip-1 x
            # rides behind them so its sweeps never wait
            load_xt_range(0, 0, 4, "xs0a")
            nc.sync.dma_start(wq_sb[:, 0:4, :], wqr[:, 0:4, :])
            nc.sync.dma_start(wk_sb[:, 0:8, :], wkr[:, 0:8, :])
            nc.sync.dma_start(wv_sb[:, 0:8, :], wvr[:, 0:8, :])
            load_xt_range(0, 4, 16, "xs0b")
            nc.sync.dma_start(wq_sb[:, 4:16, :], wqr[:, 4:16, :])
            nc.sync.dma_start(wk_sb[:, 8:32, :], wkr[:, 8:32, :])
            nc.sync.dma_start(wv_sb[:, 8:32, :], wvr[:, 8:32, :])
            load_half(0, 1)
            nc.sync.dma_start(wq_sb[:, 16:32, :], wqr[:, 16:32, :])
            load_half(1, 0)
            load_half(1, 1)

            allc = const_pool.tile([128, 4864], F16)
            nc.sync.dma_start(allc[:], const_d[:])
            ident = allc[:, 0:128]
            COS0, SIN0 = 128, 2176
            mask_sb = allc[:, 4224:4736]
            ones_sb = allc[:, 4736:4864]
            ebias = const_pool.tile([128, 1], F32)
            nc.gpsimd.memset(ebias[:], EXP_BIAS)
            # warm the ScalarE exp table during phase 1 (ACT_TABLE_LOAD is
            # ~1.3us and otherwise lands on the phase-2 critical path)
            expwarm = const_pool.tile([128, 1], F32)
            nc.scalar.activation(expwarm[:], ebias[:], AF.Exp, bias=ebias[:])

            def rope_store(src_ps, dst_ap, sslice):
                # dst = src*cos + rot(src)*sin, rot = [-src[64:], src[:64]].
                # SBUF+SBUF DVE operands must share their base partition, so
                # materialize the half-rotated src from PSUM first, then all
                # remaining ops are partition-aligned fp16 SBUF math.
                qrot = p1tmp.tile([128, SSTRIP], F16, tag="rope_qr",
                                  name="rope_qr")
                nc.vector.tensor_copy(qrot[0:64, :], src_ps[64:128, :])
                nc.vector.tensor_copy(qrot[64:128, :], src_ps[0:64, :])
                qcos = p1tmp.tile([128, SSTRIP], F16, tag="rope_qc",
                                  name="rope_qc")
                s0 = sslice.start
                nc.vector.tensor_mul(qcos[:], src_ps[:],
                                     allc[:, COS0 + s0:COS0 + s0 + SSTRIP])
                nc.vector.tensor_mul(qrot[:], qrot[:],
                                     allc[:, SIN0 + s0:SIN0 + s0 + SSTRIP])
                nc.vector.tensor_sub(dst_ap[0:64, :], qcos[0:64, :],
                                     qrot[0:64, :])
                nc.vector.tensor_add(dst_ap[64:128, :], qcos[64:128, :],
                                     qrot[64:128, :])

            def vtranspose(si, vacc):
                vstg = p1tmp.tile([128, SSTRIP], F16, tag="vstg",
                                  name="vstg")
                nc.scalar.copy(vstg[:], vacc[:])
                for ss in range(NSUB):
                    tp = tp_ps.tile([128, 128], F16, tag="tp", name="tp")
                    nc.tensor.transpose(
                        tp[:], vstg[:, ss * 128:(ss + 1) * 128], ident[:])
                    nc.vector.tensor_copy(vnat_sb[:, si * NSUB + ss, :],
                                          tp[:])

            for si in range(NSTRIPS):
                s0 = si * SSTRIP
                sslice = slice(s0, s0 + SSTRIP)
                if 1 <= si < NSTRIPS - 1:
                    # next strip's low half now; its high half at strip
                    # end (the 3-deep ring slot frees once this strip's
                    # low-half reads are all emitted)
                    load_half(si + 1, 0)
                if si == 3:
                    # prefetch wo for o_proj; deferred past the phase-1
                    # weight/x burst so it never competes with them
                    nc.sync.dma_start(wo_sb[:],
                                      wo_d.rearrange("(c p) m -> p c m",
                                                     p=128))

                qacc = [acc_ps.tile([128, SSTRIP], F32, tag=f"qacc{g}",
                                    name=f"qacc{g}")
                        for g in range(HQ)]
                kacc = acc_ps.tile([128, SSTRIP], F32, tag="kacc")
                vacc = acc_ps.tile([128, SSTRIP], F32, tag="vacc")

                xtiles = xstore.pop(si)

                def xchunk(dc):
                    for t, lo, hi in xtiles:
                        if lo <= dc < hi:
                            return t[:, dc - lo, :]
                    raise KeyError(dc)

                wsl_q = [
                    (lambda dc, g=g: wq_sb[:, dc, g * 128:(g + 1) * 128])
                    for g in range(HQ)]
                wsl_k = lambda dc: wk_sb[:, dc, :]
                wsl_v = lambda dc: wv_sb[:, dc, :]

                def mm(acc, wsl, dc):
                    nc.tensor.matmul(acc[:], wsl(dc), xchunk(dc),
                                     start=(dc == 0), stop=(dc == DCH - 1))

                if si == 0:
                    # dc-major warmup: consume x/w chunks in DMA arrival
                    # order so the PE never outruns the HBM stream during
                    # the cold start; finish output-major so the RoPE
                    # drains overlap the remaining matmuls.
                    DCSPLIT = 24
                    allacc = ([(qacc[g], wsl_q[g]) for g in range(HQ)]
                              + [(kacc, wsl_k), (vacc, wsl_v)])
                    for dc in range(DCSPLIT):
                        for acc, wsl in allacc:
                            mm(acc, wsl, dc)
                    for dc in range(DCSPLIT, DCH):
                        mm(vacc, wsl_v, dc)
                    vtranspose(si, vacc)
                    for dc in range(DCSPLIT, DCH):
                        mm(kacc, wsl_k, dc)
                    rope_store(kacc, kT_t[si][:], sslice)
                    for g in reversed(range(HQ)):
                        for dc in range(DCSPLIT, DCH):
                            mm(qacc[g], wsl_q[g], dc)
                        rope_store(qacc[g], qT_t[si][:, g, :], sslice)
                else:
                    # dc-major over the low half: all six accumulators
                    # advance together, so the strip's high-half x tile
                    # (whose DMA was issued only at the previous strip's
                    # end) isn't touched until ~21us into the strip.
                    # Then output-major k, v(+transposes), q3..q0 over the
                    # high half with drains trailing under later sweeps.
                    allacc = ([(kacc, wsl_k), (vacc, wsl_v)]
                              + [(qacc[g], wsl_q[g])
                                 for g in reversed(range(HQ))])
                    for dc in range(16):
                        for acc, wsl in allacc:
                            mm(acc, wsl, dc)
                    for dc in range(16, DCH):
                        mm(kacc, wsl_k, dc)
                    rope_store(kacc, kT_t[si][:], sslice)
                    if si < NSTRIPS - 1:
                        for dc in range(16, DCH):
                            mm(vacc, wsl_v, dc)
                        vtranspose(si, vacc)
                    for g in reversed(range(HQ)):
                        for dc in range(16, DCH):
                            mm(qacc[g], wsl_q[g], dc)
                        rope_store(qacc[g], qT_t[si][:, g, :], sslice)
                    if si == NSTRIPS - 1:
                        # v last: its 16 matmuls + transposes are TensorE
                        # work that covers all four q RoPE drains, so no
                        # phase-2 PSUM bank waits on a drain's reads
                        for dc in range(16, DCH):
                            mm(vacc, wsl_v, dc)
                        vtranspose(si, vacc)
                    if 1 <= si < NSTRIPS - 1:
                        load_half(si + 1, 1)

        # -------- phase 2: attention + o_proj, one pipelined stream --------
        # job = (qi, h, pp): one pair of key chunks (2*128 keys) against
        # q-tile qi (256 queries) for head h. npairs(qi) = qi+1.
        jobs = []
        for qi in range(NQT):
            for h in range(HQ):
                npairs = qi + 1
                for pp in range(npairs):
                    jobs.append((qi, h, pp, npairs))
        njobs = len(jobs)

        with tc.tile_pool(name="pt", bufs=4) as pt_pool, \
             tc.tile_pool(name="pacc", bufs=6) as pacc_pool, \
             tc.tile_pool(name="a2tmp", bufs=2) as a2tmp, \
             tc.tile_pool(name="osb", bufs=2) as osb_pool, \
             tc.tile_pool(name="st_ps", bufs=ST_BUFS, space="PSUM") as st_ps, \
             tc.tile_pool(name="oacc_ps", bufs=2, space="PSUM") as oacc_ps, \
             tc.tile_pool(name="rb_ps", bufs=1, space="PSUM") as rb_ps, \
             tc.tile_pool(name="opj_ps", bufs=2, space="PSUM") as opj_ps:

            st_tiles = {}    # j -> score PSUM tile
            pt_tiles = {}    # j -> exp'd prob tile (SBUF fp16)
            blk_state = {}   # (qi,h) -> dict(pacc=, oacc=)
            pending_tails = deque()   # ((qi,h), emit_at_job)
            pending_oproj = deque()   # (si, mi)
            osb_tiles = {}   # si -> osb tile

            def emit_scores(j):
                qi, h, pp, npairs = jobs[j]
                q0 = qi * QTILE
                st = st_ps.tile([128, 2 * QTILE], F32, tag="st", name="st")
                ql = (qi % 2) * QTILE
                for c in range(2):
                    pi = 2 * pp + c
                    kl = (pi % 4) * 128
                    nc.tensor.matmul(
                        st[:, c * QTILE:(c + 1) * QTILE],
                        kT_t[pi // 4][:, kl:kl + 128],
                        qT_t[qi // 2][:, h, ql:ql + QTILE],
                        start=True, stop=True)
                st_tiles[j] = st

            def emit_exp_acc(j):
                qi, h, pp, npairs = jobs[j]
                pt = pt_pool.tile([128, 2 * QTILE], F16, tag="pt", name="pt")
                nc.scalar.activation(pt[:], st_tiles.pop(j)[:], AF.Exp,
                                     bias=ebias[:])
                if pp == npairs - 1:
                    # final pair straddles the causal diagonal
                    nc.vector.tensor_mul(pt[:], pt[:], mask_sb[:])
                pt_tiles[j] = pt
                # denominator accumulation on DVE: one double-width add per
                # pair (even chunks land in cols [0,256), odd in [256,512));
                # the fold to per-query sums happens via the rb matmul + one
                # narrow add in the tail. Halves the serial chain per block.
                stt = blk_state[(qi, h)]
                if pp == 0:
                    pacc = pacc_pool.tile([128, 2 * QTILE], F16, tag="pacc",
                                          name="pacc")
                    nc.vector.tensor_copy(pacc[:], pt[:])
                    stt["pacc"] = pacc
                else:
                    pacc = stt["pacc"]
                    nc.vector.tensor_add(pacc[:], pacc[:], pt[:])

            def emit_pv(j):
                qi, h, pp, npairs = jobs[j]
                stt = blk_state[(qi, h)]
                if pp == 0:
                    stt["oacc"] = oacc_ps.tile([128, QTILE], F32, tag="oacc", name="oacc")
                oacc = stt["oacc"]
                pt = pt_tiles.pop(j)
                for c in range(2):
                    pi = 2 * pp + c
                    nc.tensor.matmul(
                        oacc[:], vnat_sb[:, pi, :],
                        pt[:, c * QTILE:(c + 1) * QTILE],
                        start=(pi == 0), stop=(pi == 2 * npairs - 1))

            def emit_tail(key):
                qi, h = key
                stt = blk_state.pop(key)
                q0 = qi * QTILE
                # two matmuls against an all-ones stationary reduce pacc
                # over partitions, fold the even/odd-chunk halves via PSUM
                # accumulation, and broadcast the sums to all 128 output
                # partitions
                rb = rb_ps.tile([128, QTILE], F32, tag="rb", name="rb")
                nc.tensor.matmul(rb[:], ones_sb[:], stt["pacc"][:, 0:QTILE],
                                 start=True, stop=False)
                nc.tensor.matmul(rb[:], ones_sb[:],
                                 stt["pacc"][:, QTILE:2 * QTILE],
                                 start=False, stop=True)
                rbr = a2tmp.tile([128, QTILE], F32, tag="rbr", name="rbr")
                nc.vector.reciprocal_approx_fast(rbr[:], rb[:])
                nc.vector.tensor_mul(outhT_sb[:, h, q0:q0 + QTILE],
                                     stt["oacc"][:], rbr[:])
                if h == HQ - 1:
                    for si in (2 * qi, 2 * qi + 1):
                        for mi in range(D // 512):
                            pending_oproj.append((si, mi))

            def emit_oproj_block():
                si, mi = pending_oproj.popleft()
                if mi == 0:
                    osb_tiles[si] = osb_pool.tile([128, D], F16, tag="osb",
                                                  name="osb")
                osb = osb_tiles[si]
                op = opj_ps.tile([128, 512], F32, tag="opj", name="opj")
                for h in range(HQ):
                    nc.tensor.matmul(
                        op[:],
                        outhT_sb[:, h, si * 128:(si + 1) * 128],
                        wo_sb[:, h, mi * 512:(mi + 1) * 512],
                        start=(h == 0), stop=(h == HQ - 1))
                if mi % 2 == 0:
                    nc.vector.tensor_copy(
                        osb[:, mi * 512:(mi + 1) * 512], op[:])
                else:
                    nc.scalar.copy(
                        osb[:, mi * 512:(mi + 1) * 512], op[:])
                # one output DMA per row-block (DMA triggers are ~1us of
                # serial Sync time each); the last block goes in halves so
                # its DMA overlaps the final copies
                if si == 2 * NQT - 1:
                    if mi == 3:
                        nc.sync.dma_start(
                            out_d[si * 128:(si + 1) * 128, 0:2048],
                            osb[:, 0:2048])
                    elif mi == 7:
                        half = osb_tiles.pop(si)
                        nc.sync.dma_start(
                            out_d[si * 128:(si + 1) * 128, 2048:4096],
                            half[:, 2048:4096])
                elif mi == 7:
                    full = osb_tiles.pop(si)
                    nc.sync.dma_start(out_d[si * 128:(si + 1) * 128, :],
                                      full[:])

            for j in range(njobs + PIPE_LAG):
                if j < njobs:
                    qi, h, pp, npairs = jobs[j]
                    if pp == 0:
                        blk_state[(qi, h)] = {}
                    emit_scores(j)
                    emit_exp_acc(j)
                jl = j - PIPE_LAG
                if jl >= 0:
                    qi, h, pp, npairs = jobs[jl]
                    emit_pv(jl)
                    if pp == npairs - 1:
                        due = j + (2 if h == HQ - 1 else 1)
                        pending_tails.append(((qi, h), due))
                if pending_oproj and (len(pending_oproj) > 4
                                      or j % 2 == 0):
                    # pace the drain so dense o_proj work lasts through the
                    # ScalarE-limited attention-only stretches
                    emit_oproj_block()
                while pending_tails and pending_tails[0][1] <= j:
                    emit_tail(pending_tails.popleft()[0])
            # drain remaining tails and o_proj blocks
            while pending_tails:
                emit_tail(pending_tails.popleft()[0])
            while pending_oproj:
                emit_oproj_block()
        outh_pool_cm.__exit__(None, None, None)
        wo_pool_cm.__exit__(None, None, None)


_NC_CACHE = None
LAST_RESULT = None
RUN_KWARGS = {}


def _get_nc():
    global _NC_CACHE
    if _NC_CACHE is None:
        _NC_CACHE = build()
    return _NC_CACHE


def kernel(x, wq, wk, wv, wo):
    global LAST_RESULT
    x = np.asarray(x, dtype=np.float32).reshape(S, D)
    xt = np.ascontiguousarray(x.T.astype(np.float16))
    wq = (np.asarray(wq, dtype=np.float32)
          * np.float32(1.0 / np.sqrt(HD))).astype(np.float16)
    wk = np.asarray(wk, dtype=np.float32).astype(np.float16)
    wv = np.asarray(wv, dtype=np.float32).astype(np.float16)
    wo = np.asarray(wo, dtype=np.float32).astype(np.float16)

    in_maps = []
    for c in range(NCORES):
        in_maps.append({
            "xt": xt,
            "wq": np.ascontiguousarray(wq[:, c * NQ:(c + 1) * NQ]),
            "wk": np.ascontiguousarray(wk[:, c * NKV:(c + 1) * NKV]),
            "wv": np.ascontiguousarray(wv[:, c * NKV:(c + 1) * NKV]),
            "wo": np.ascontiguousarray(wo[c * NQ:(c + 1) * NQ, :]),
        })

    nc = _get_nc()
    res = bass_utils.run_bass_kernel_spmd(nc, in_maps,
                                          core_ids=list(range(NCORES)),
                                          **RUN_KWARGS)
    LAST_RESULT = res
    acc = np.zeros((S, D), dtype=np.float64)
    for c in range(NCORES):
        acc += res.results[c]["out"].astype(np.float64)
    return acc.astype(np.float32).reshape(1, S, D)


# revision 27
# speedup vs baseline: 1.1905x; 1.0044x over previous
"""Trainium2 Bass kernel for GQA attention with RoPE (tensor-parallel over heads).

Reference computation (per problem spec):
  x:[1,2048,4096], wq:[4096,4096], wk/wv:[4096,1024], wo:[4096,4096], f32
  q/k/v proj -> RoPE(q,k) -> causal GQA softmax attention -> o_proj

Sharding: 8 cores, tensor-parallel over heads. Core c gets 4 query heads
(wq cols [c*512:(c+1)*512]) and 1 KV head (wk/wv cols [c*128:(c+1)*128]),
plus wo rows [c*512:(c+1)*512]. Each core computes a full [2048,4096]
partial o_proj output; the host sums the 8 partials (the all-reduce).
The host dispatch layer hands the device x pre-transposed ([D,S]) -- the
TensorE contracts over the partition axis, so both matmul operands need
d on partitions.

Matmul operands are fp16 (see previous-gen docstring: FWL hides weight
loads under the 1-col/cycle matmul stream; fp32 pays serialized 2-pass
LDWEIGHTS, fp8 DoubleRow fails the 2e-2 accuracy gate by 2-5x). All
accumulation is fp32 in PSUM.

Phase 2 is a single flattened software-pipelined job stream: each job is
one (q-tile, head, key-chunk-pair) -> 2 score MMs [128x128x256] into one
PSUM bank, one exp ACT, (diagonal-only) causal mask multiply on DVE, and
2 PV MMs lagging the scores by PIPE_LAG jobs so the exp latency hides
under later jobs' score MMs. Softmax denominators are accumulated on
VectorE (pacc += exp chunks, fp16) instead of the old per-chunk
ones-vector matmuls (which cost 295ns each of pure TensorE time and
broke FWL for the neighboring PV matmuls); one [128x128x256] matmul
against an all-ones stationary both reduces pacc over partitions and
broadcasts the result to all 128 partitions, replacing the old
GpSimd partition_broadcast. o_proj mi-blocks are interleaved one per
job into the stream so TensorE always has dense independent work while
ScalarE catches up on exps.
"""
import numpy as np
from collections import deque

import concourse.bass as bass
import concourse.bacc as bacc
import concourse.tile as tile
import concourse.mybir as mybir
from concourse import bass_utils

F32 = mybir.dt.float32
F16 = mybir.dt.float16
AF = mybir.ActivationFunctionType

# model dims (hardcoded per problem spec nn_Attention_52020643889298)
S = 2048
D = 4096
H = 32
KV = 8
HD = 128
THETA = 10000.0
NCORES = 8
HQ = H // NCORES            # 4 query heads per core
NQ = HQ * HD                # 512 wq cols per core
NKV = (KV // NCORES) * HD   # 128 wk/wv cols per core

# tiling
SSTRIP = 512                # phase-1 s-strip
NSTRIPS = S // SSTRIP       # 4
NSUB = SSTRIP // 128        # 4
DCH = D // 128              # 32 contraction chunks
QTILE = 256                 # attention q-tile
NQT = S // QTILE            # 8
NPCH = S // 128             # 16 key chunks

EXP_BIAS = -10.0            # exp(s-10): keeps exp in fp16 range; cancels
                            # in the softmax normalization
PIPE_LAG = 2                # PV lags scores by this many jobs
ST_BUFS = 3                 # score PSUM ring (1 bank each at QTILE=256)


def _rope_tables():
    inv = 1.0 / (THETA ** (np.arange(0, HD, 2, dtype=np.float64) / HD))
    pos = np.arange(S, dtype=np.float64)
    freqs = pos[:, None] * inv[None, :]          # [S, 64]
    emb = np.concatenate([freqs, freqs], axis=1)  # [S, HD]
    cosT = np.cos(emb).T.astype(np.float16).copy()  # [HD, S]
    sinT = np.sin(emb).T.astype(np.float16).copy()
    return cosT, sinT


def _mask_pair():
    # One key-chunk pair (256 keys) against QTILE queries on the diagonal:
    # chunk 0 keys are at local offset p, chunk 1 at p+128.
    # mask[p, c*QTILE + q'] = 1 iff q' - p >= 128*c
    q = np.arange(QTILE)[None, :]
    p = np.arange(128)[:, None]
    cols = [np.where(q - p >= 128 * c, 1.0, 0.0) for c in range(2)]
    return np.concatenate(cols, axis=1).astype(np.float16)  # [128, 512]


def build():
    nc = bacc.Bacc("TRN2", target_bir_lowering=False, debug=False,
                   enable_asserts=False, num_devices=NCORES)
    xt_d = nc.dram_tensor("xt", [D, S], F16, kind="ExternalInput").ap()
    wq_d = nc.dram_tensor("wq", [D, NQ], F16, kind="ExternalInput").ap()
    wk_d = nc.dram_tensor("wk", [D, NKV], F16, kind="ExternalInput").ap()
    wv_d = nc.dram_tensor("wv", [D, NKV], F16, kind="ExternalInput").ap()
    wo_d = nc.dram_tensor("wo", [NQ, D], F16, kind="ExternalInput").ap()
    out_d = nc.dram_tensor("out", [S, D], F16, kind="ExternalOutput").ap()

    cosT, sinT = _rope_tables()
    allconst = np.concatenate(
        [np.eye(128, dtype=np.float16), cosT, sinT, _mask_pair(),
         np.ones((128, 128), dtype=np.float16)], axis=1)
    const_d = nc.inline_tensor(allconst, "allconst").ap()

    with tile.TileContext(nc) as tc:
        _body(nc, tc, xt_d, wq_d, wk_d, wv_d, wo_d, out_d, const_d)
    nc.compile()
    return nc


def _body(nc, tc, xt_d, wq_d, wk_d, wv_d, wo_d, out_d, const_d):
    wqr = wq_d.rearrange("(p c) n -> p c n", p=128)
    wkr = wk_d.rearrange("(p c) n -> p c n", p=128)
    wvr = wv_d.rearrange("(p c) n -> p c n", p=128)

    with tc.tile_pool(name="const", bufs=1) as const_pool, \
         tc.tile_pool(name="persist", bufs=1) as persist:

        # persistent activations; qT/kT are split per strip so phase-2's
        # first jobs (which only need strip 0) don't inherit a dependency
        # on the last strip's RoPE drains
        qT_t = [persist.tile([128, HQ, SSTRIP], F16, name=f"qT{i}")
                for i in range(NSTRIPS)]
        kT_t = [persist.tile([128, SSTRIP], F16, name=f"kT{i}")
                for i in range(NSTRIPS)]
        vnat_sb = persist.tile([128, NPCH, HD], F16)  # [s%128, s//128, hd]

        # ---------------- phase 1: QKV projection + RoPE ----------------
        wo_pool_cm = tc.tile_pool(name="wo2", bufs=1)
        outh_pool_cm = tc.tile_pool(name="outh", bufs=1)
        wo_pool = wo_pool_cm.__enter__()
        outh_pool = outh_pool_cm.__enter__()
        wo_sb = wo_pool.tile([128, HQ, D], F16)
        outhT_sb = outh_pool.tile([128, HQ, S], F16)  # [hd, head, s]
        with tc.tile_pool(name="w1", bufs=1) as w1, \
             tc.tile_pool(name="xt", bufs=1) as xt_pool, \
             tc.tile_pool(name="p1tmp", bufs=2) as p1tmp, \
             tc.tile_pool(name="tp_ps", bufs=2, space="PSUM") as tp_ps, \
             tc.tile_pool(name="acc_ps", bufs=1, space="PSUM") as acc_ps:

            wq_sb = w1.tile([128, DCH, NQ], F16)
            wk_sb = w1.tile([128, DCH, NKV], F16)
            wv_sb = w1.tile([128, DCH, NKV], F16)

            xtr = xt_d.rearrange("(p c) s -> p c s", p=128)  # [128, DCH, S]

            # Every dma_start costs the Sync engine ~1us of serial SWDGE
            # descriptor-generation time regardless of size, so DMAs are
            # few and large, issued in consumption order: strip-0 x and wq
            # arrive as graduated ranges (small first for latency), wk/wv
            # in two pieces, later strips as two half-strip tiles
            # prefetched one strip ahead.
            xstore = {}   # si -> list of (tile, lo, hi)

            def load_xt_range(si, lo, hi, tag, bufs=1):
                t = xt_pool.tile([128, hi - lo, SSTRIP], F16, tag=tag,
                                 name=f"xt{si}_{lo}", bufs=bufs)
                nc.sync.dma_start(
                    t[:], xtr[:, lo:hi,
                              si * SSTRIP:(si + 1) * SSTRIP])
                xstore.setdefault(si, []).append((t, lo, hi))

            def load_half(si, half):
                load_xt_range(si, 16 * half, 16 * half + 16, "xtbig", bufs=3)

            # interleave strip-0 x and weights in dc order; strip-1 x
            # rides behind them so its sweeps never wait
            load_xt_range(0, 0, 4, "xs0a")
            nc.sync.dma_start(wq_sb[:, 0:4, :], wqr[:, 0:4, :])
            nc.sync.dma_start(wk_sb[:, 0:8, :], wkr[:, 0:8, :])
            nc.sync.dma_start(wv_sb[:, 0:8, :], wvr[:, 0:8, :])
            load_xt_range(0, 4, 10, "xs0b")
            nc.sync.dma_start(wq_sb[:, 4:16, :], wqr[:, 4:16, :])
            load_xt_range(0, 10, 16, "xs0c")
            nc.sync.dma_start(wk_sb[:, 8:32, :], wkr[:, 8:32, :])
            nc.sync.dma_start(wv_sb[:, 8:32, :], wvr[:, 8:32, :])
            load_half(0, 1)
            nc.sync.dma_start(wq_sb[:, 16:32, :], wqr[:, 16:32, :])
            load_half(1, 0)
            load_half(1, 1)

            allc = const_pool.tile([128, 4864], F16)
            nc.sync.dma_start(allc[:], const_d[:])
            ident = allc[:, 0:128]
            COS0, SIN0 = 128, 2176
            mask_sb = allc[:, 4224:4736]
            ones_sb = allc[:, 4736:4864]
            ebias = const_pool.tile([128, 1], F32)
            nc.gpsimd.memset(ebias[:], EXP_BIAS)
            # warm the ScalarE exp table during phase 1 (ACT_TABLE_LOAD is
            # ~1.3us and otherwise lands on the phase-2 critical path)
            expwarm = const_pool.tile([128, 1], F32)
            nc.scalar.activation(expwarm[:], ebias[:], AF.Exp, bias=ebias[:])

            def rope_store(src_ps, dst_ap, sslice):
                # dst = src*cos + rot(src)*sin, rot = [-src[64:], src[:64]].
                # SBUF+SBUF DVE operands must share their base partition, so
                # materialize the half-rotated src from PSUM first, then all
                # remaining ops are partition-aligned fp16 SBUF math.
                qrot = p1tmp.tile([128, SSTRIP], F16, tag="rope_qr",
                                  name="rope_qr")
                nc.vector.tensor_copy(qrot[0:64, :], src_ps[64:128, :])
                nc.vector.tensor_copy(qrot[64:128, :], src_ps[0:64, :])
                qcos = p1tmp.tile([128, SSTRIP], F16, tag="rope_qc",
                                  name="rope_qc")
                s0 = sslice.start
                nc.vector.tensor_mul(qcos[:], src_ps[:],
                                     allc[:, COS0 + s0:COS0 + s0 + SSTRIP])
                nc.vector.tensor_mul(qrot[:], qrot[:],
                                     allc[:, SIN0 + s0:SIN0 + s0 + SSTRIP])
                nc.vector.tensor_sub(dst_ap[0:64, :], qcos[0:64, :],
                                     qrot[0:64, :])
                nc.vector.tensor_add(dst_ap[64:128, :], qcos[64:128, :],
                                     qrot[64:128, :])

            def vtranspose(si, vacc):
                vstg = p1tmp.tile([128, SSTRIP], F16, tag="vstg",
                                  name="vstg")
                nc.scalar.copy(vstg[:], vacc[:])
                for ss in range(NSUB):
                    tp = tp_ps.tile([128, 128], F16, tag="tp", name="tp")
                    nc.tensor.transpose(
                        tp[:], vstg[:, ss * 128:(ss + 1) * 128], ident[:])
                    nc.vector.tensor_copy(vnat_sb[:, si * NSUB + ss, :],
                                          tp[:])

            for si in range(NSTRIPS):
                s0 = si * SSTRIP
                sslice = slice(s0, s0 + SSTRIP)
                if 1 <= si < NSTRIPS - 1:
                    # next strip's low half now; its high half at strip
                    # end (the 3-deep ring slot frees once this strip's
                    # low-half reads are all emitted)
                    load_half(si + 1, 0)
                if si == 3:
                    # prefetch wo for o_proj; deferred past the phase-1
                    # weight/x burst so it never competes with them
                    nc.sync.dma_start(wo_sb[:],
                                      wo_d.rearrange("(c p) m -> p c m",
                                                     p=128))

                qacc = [acc_ps.tile([128, SSTRIP], F32, tag=f"qacc{g}",
                                    name=f"qacc{g}")
                        for g in range(HQ)]
                kacc = acc_ps.tile([128, SSTRIP], F32, tag="kacc")
                vacc = acc_ps.tile([128, SSTRIP], F32, tag="vacc")

                xtiles = xstore.pop(si)

                def xchunk(dc):
                    for t, lo, hi in xtiles:
                        if lo <= dc < hi:
                            return t[:, dc - lo, :]
                    raise KeyError(dc)

                wsl_q = [
                    (lambda dc, g=g: wq_sb[:, dc, g * 128:(g + 1) * 128])
                    for g in range(HQ)]
                wsl_k = lambda dc: wk_sb[:, dc, :]
                wsl_v = lambda dc: wv_sb[:, dc, :]

                def mm(acc, wsl, dc):
                    nc.tensor.matmul(acc[:], wsl(dc), xchunk(dc),
                                     start=(dc == 0), stop=(dc == DCH - 1))

                if si == 0:
                    # dc-major warmup: consume x/w chunks in DMA arrival
                    # order so the PE never outruns the HBM stream during
                    # the cold start; finish output-major so the RoPE
                    # drains overlap the remaining matmuls.
                    DCSPLIT = 24
                    allacc = ([(qacc[g], wsl_q[g]) for g in range(HQ)]
                              + [(kacc, wsl_k), (vacc, wsl_v)])
                    for dc in range(DCSPLIT):
                        for acc, wsl in allacc:
                            mm(acc, wsl, dc)
                    for dc in range(DCSPLIT, DCH):
                        mm(vacc, wsl_v, dc)
                    vtranspose(si, vacc)
                    for dc in range(DCSPLIT, DCH):
                        mm(kacc, wsl_k, dc)
                    rope_store(kacc, kT_t[si][:], sslice)
                    for g in reversed(range(HQ)):
                        for dc in range(DCSPLIT, DCH):
                            mm(qacc[g], wsl_q[g], dc)
                        rope_store(qacc[g], qT_t[si][:, g, :], sslice)
                else:
                    # dc-major over the low half: all six accumulators
                    # advance together, so the strip's high-half x tile
                    # (whose DMA was issued only at the previous strip's
                    # end) isn't touched until ~21us into the strip.
                    # Then output-major k, v(+transposes), q3..q0 over the
                    # high half with drains trailing under later sweeps.
                    allacc = ([(kacc, wsl_k), (vacc, wsl_v)]
                              + [(qacc[g], wsl_q[g])
                                 for g in reversed(range(HQ))])
                    for dc in range(16):
                        for acc, wsl in allacc:
                            mm(acc, wsl, dc)
                    for dc in range(16, DCH):
                        mm(kacc, wsl_k, dc)
                    rope_store(kacc, kT_t[si][:], sslice)
                    if si < NSTRIPS - 1:
                        for dc in range(16, DCH):
                            mm(vacc, wsl_v, dc)
                        vtranspose(si, vacc)
                    for g in reversed(range(HQ)):
                        for dc in range(16, DCH):
                            mm(qacc[g], wsl_q[g], dc)
                        rope_store(qacc[g], qT_t[si][:, g, :], sslice)
                    if si == NSTRIPS - 1:
                        # v last: its 16 matmuls + transposes are TensorE
                        # work that covers all four q RoPE drains, so no
                        # phase-2 PSUM bank waits on a drain's reads
                        for dc in range(16, DCH):
                            mm(vacc, wsl_v, dc)
                        vtranspose(si, vacc)
                    if 1 <= si < NSTRIPS - 1:
                        load_half(si + 1, 1)

        # -------- phase 2: attention + o_proj, one pipelined stream --------
        # job = (qi, h, pp): one pair of key chunks (2*128 keys) against
        # q-tile qi (256 queries) for head h. npairs(qi) = qi+1.
        jobs = []
        for qi in range(NQT):
            for h in range(HQ):
                npairs = qi + 1
                for pp in range(npairs):
                    jobs.append((qi, h, pp, npairs))
        njobs = len(jobs)

        with tc.tile_pool(name="pt", bufs=4) as pt_pool, \
             tc.tile_pool(name="pacc", bufs=6) as pacc_pool, \
             tc.tile_pool(name="a2tmp", bufs=2) as a2tmp, \
             tc.tile_pool(name="osb", bufs=2) as osb_pool, \
             tc.tile_pool(name="st_ps", bufs=ST_BUFS, space="PSUM") as st_ps, \
             tc.tile_pool(name="oacc_ps", bufs=2, space="PSUM") as oacc_ps, \
             tc.tile_pool(name="rb_ps", bufs=1, space="PSUM") as rb_ps, \
             tc.tile_pool(name="opj_ps", bufs=2, space="PSUM") as opj_ps:

            st_tiles = {}    # j -> score PSUM tile
            pt_tiles = {}    # j -> exp'd prob tile (SBUF fp16)
            blk_state = {}   # (qi,h) -> dict(pacc=, oacc=)
            pending_tails = deque()   # ((qi,h), emit_at_job)
            pending_oproj = deque()   # (si, mi)
            osb_tiles = {}   # si -> osb tile

            def emit_scores(j):
                qi, h, pp, npairs = jobs[j]
                q0 = qi * QTILE
                st = st_ps.tile([128, 2 * QTILE], F32, tag="st", name="st")
                ql = (qi % 2) * QTILE
                for c in range(2):
                    pi = 2 * pp + c
                    kl = (pi % 4) * 128
                    nc.tensor.matmul(
                        st[:, c * QTILE:(c + 1) * QTILE],
                        kT_t[pi // 4][:, kl:kl + 128],
                        qT_t[qi // 2][:, h, ql:ql + QTILE],
                        start=True, stop=True)
                st_tiles[j] = st

            def emit_exp_acc(j):
                qi, h, pp, npairs = jobs[j]
                pt = pt_pool.tile([128, 2 * QTILE], F16, tag="pt", name="pt")
                nc.scalar.activation(pt[:], st_tiles.pop(j)[:], AF.Exp,
                                     bias=ebias[:])
                if pp == npairs - 1:
                    # final pair straddles the causal diagonal
                    nc.vector.tensor_mul(pt[:], pt[:], mask_sb[:])
                pt_tiles[j] = pt
                # denominator accumulation on DVE: one double-width add per
                # pair (even chunks land in cols [0,256), odd in [256,512));
                # the fold to per-query sums happens via the rb matmul + one
                # narrow add in the tail. Halves the serial chain per block.
                stt = blk_state[(qi, h)]
                if pp == 0:
                    pacc = pacc_pool.tile([128, 2 * QTILE], F16, tag="pacc",
                                          name="pacc")
                    nc.vector.tensor_copy(pacc[:], pt[:])
                    stt["pacc"] = pacc
                else:
                    pacc = stt["pacc"]
                    nc.vector.tensor_add(pacc[:], pacc[:], pt[:])

            def emit_pv(j):
                qi, h, pp, npairs = jobs[j]
                stt = blk_state[(qi, h)]
                if pp == 0:
                    stt["oacc"] = oacc_ps.tile([128, QTILE], F32, tag="oacc", name="oacc")
                oacc = stt["oacc"]
                pt = pt_tiles.pop(j)
                for c in range(2):
                    pi = 2 * pp + c
                    nc.tensor.matmul(
                        oacc[:], vnat_sb[:, pi, :],
                        pt[:, c * QTILE:(c + 1) * QTILE],
                        start=(pi == 0), stop=(pi == 2 * npairs - 1))

            def emit_tail(key):
                qi, h = key
                stt = blk_state.pop(key)
                q0 = qi * QTILE
                # two matmuls against an all-ones stationary reduce pacc
                # over partitions, fold the even/odd-chunk halves via PSUM
                # accumulation, and broadcast the sums to all 128 output
                # partitions
                rb = rb_ps.tile([128, QTILE], F32, tag="rb", name="rb")
                nc.tensor.matmul(rb[:], ones_sb[:], stt["pacc"][:, 0:QTILE],
                                 start=True, stop=False)
                nc.tensor.matmul(rb[:], ones_sb[:],
                                 stt["pacc"][:, QTILE:2 * QTILE],
                                 start=False, stop=True)
                rbr = a2tmp.tile([128, QTILE], F32, tag="rbr", name="rbr")
                nc.vector.reciprocal_approx_fast(rbr[:], rb[:])
                nc.vector.tensor_mul(outhT_sb[:, h, q0:q0 + QTILE],
                                     stt["oacc"][:], rbr[:])
                if h == HQ - 1:
                    for si in (2 * qi, 2 * qi + 1):
                        for mi in range(D // 512):
                            pending_oproj.append((si, mi))

            def emit_oproj_block():
                si, mi = pending_oproj.popleft()
                if mi == 0:
                    osb_tiles[si] = osb_pool.tile([128, D], F16, tag="osb",
                                                  name="osb")
                osb = osb_tiles[si]
                op = opj_ps.tile([128, 512], F32, tag="opj", name="opj")
                for h in range(HQ):
                    nc.tensor.matmul(
                        op[:],
                        outhT_sb[:, h, si * 128:(si + 1) * 128],
                        wo_sb[:, h, mi * 512:(mi + 1) * 512],
                        start=(h == 0), stop=(h == HQ - 1))
                if mi % 2 == 0:
                    nc.vector.tensor_copy(
                        osb[:, mi * 512:(mi + 1) * 512], op[:])
                else:
                    nc.scalar.copy(
                        osb[:, mi * 512:(mi + 1) * 512], op[:])
                # one output DMA per row-block (DMA triggers are ~1us of
                # serial Sync time each); the last block goes in halves so
                # its DMA overlaps the final copies
                if si == 2 * NQT - 1:
                    if mi == 3:
                        nc.sync.dma_start(
                            out_d[si * 128:(si + 1) * 128, 0:2048],
                            osb[:, 0:2048])
                    elif mi == 5:
                        nc.sync.dma_start(
                            out_d[si * 128:(si + 1) * 128, 2048:3072],
                            osb[:, 2048:3072])
                    elif mi == 7:
                        quart = osb_tiles.pop(si)
                        nc.sync.dma_start(
                            out_d[si * 128:(si + 1) * 128, 3072:4096],
                            quart[:, 3072:4096])
                elif mi == 7:
                    full = osb_tiles.pop(si)
                    nc.sync.dma_start(out_d[si * 128:(si + 1) * 128, :],
                                      full[:])

            for j in range(njobs + PIPE_LAG):
                if j < njobs:
                    qi, h, pp, npairs = jobs[j]
                    if pp == 0:
                        blk_state[(qi, h)] = {}
                    emit_scores(j)
                    emit_exp_acc(j)
                jl = j - PIPE_LAG
                if jl >= 0:
                    qi, h, pp, npairs = jobs[jl]
                    emit_pv(jl)
                    if pp == npairs - 1:
                        due = j + (2 if h == HQ - 1 else 1)
                        pending_tails.append(((qi, h), due))
                if pending_oproj and (len(pending_oproj) > 4
                                      or j % 2 == 0):
                    # pace the drain so dense o_proj work lasts through the
                    # ScalarE-limited attention-only stretches
                    emit_oproj_block()
                while pending_tails and pending_tails[0][1] <= j:
                    emit_tail(pending_tails.popleft()[0])
            # drain remaining tails and o_proj blocks
            while pending_tails:
                emit_tail(pending_tails.popleft()[0])
            while pending_oproj:
                emit_oproj_block()
        outh_pool_cm.__exit__(None, None, None)
        wo_pool_cm.__exit__(None, None, None)


_NC_CACHE = None
LAST_RESULT = None
RUN_KWARGS = {}


def _get_nc():
    global _NC_CACHE
    if _NC_CACHE is None:
        _NC_CACHE = build()
    return _NC_CACHE


def kernel(x, wq, wk, wv, wo):
    global LAST_RESULT
    x = np.asarray(x, dtype=np.float32).reshape(S, D)
    xt = np.ascontiguousarray(x.T.astype(np.float16))
    wq = (np.asarray(wq, dtype=np.float32)
          * np.float32(1.0 / np.sqrt(HD))).astype(np.float16)
    wk = np.asarray(wk, dtype=np.float32).astype(np.float16)
    wv = np.asarray(wv, dtype=np.float32).astype(np.float16)
    wo = np.asarray(wo, dtype=np.float32).astype(np.float16)

    in_maps = []
    for c in range(NCORES):
        in_maps.append({
            "xt": xt,
            "wq": np.ascontiguousarray(wq[:, c * NQ:(c + 1) * NQ]),
            "wk": np.ascontiguousarray(wk[:, c * NKV:(c + 1) * NKV]),
            "wv": np.ascontiguousarray(wv[:, c * NKV:(c + 1) * NKV]),
            "wo": np.ascontiguousarray(wo[c * NQ:(c + 1) * NQ, :]),
        })

    nc = _get_nc()
    res = bass_utils.run_bass_kernel_spmd(nc, in_maps,
                                          core_ids=list(range(NCORES)),
                                          **RUN_KWARGS)
    LAST_RESULT = res
    acc = np.zeros((S, D), dtype=np.float64)
    for c in range(NCORES):
        acc += res.results[c]["out"].astype(np.float64)
    return acc.astype(np.float32).reshape(1, S, D)
